# revision 11
# baseline (speedup 1.0000x reference)
"""Multi-head attention forward kernel for Trainium2 (8 NeuronCores).

Problem: B=2, N=2048, C=1024, H=16 heads, head_dim=64.
    q = x @ Wq.T + bq  (same for k, v)
    out = softmax(q k^T / sqrt(C)) v       (per head), re-merged to [B, N, C]

Sharding: core = (batch b, head-group g): b = core // 4, g = core % 4.
Each core computes 4 heads of one batch element. No collectives needed --
outputs are disjoint; host gathers and finishes with a cheap epilogue
(normalize by the row-sums, add bv, transpose).

Measured engine economics (HW trace):
  - PE matmul cost = out-free-rows x 0.42ns; packed pairs (row/col
    tile_position) run concurrently at ~0.65ns/row combined.
  - ACT exp ~1.0ns/elem; DVE fp16 add 0.6ns/elem, PSUM-touching 1.1ns/elem.
  - QK+PV are at the bf16 PE floor (~85us); projections ~45us; exp 131us.
Span is bound by ACT's 131us exp stream + PE's ~150us busy; the job of the
emission schedule is to keep BOTH fed: V-projection is split per head-pair
(pair-1's half deferred out of the overloaded first query block), q/k
projection blocks are spread as PE filler into windows where the PE would
otherwise idle behind the exp stream.
"""

import os
import sys

import ml_dtypes
import numpy as np

for _p in ("/opt/trn_rl_repo",):
    if _p not in sys.path:
        sys.path.insert(0, _p)

import concourse.bass as bass  # noqa: E402
import concourse.tile as tile  # noqa: E402
from concourse import bacc, mybir  # noqa: E402
from concourse.bass_utils import run_bass_kernel_spmd  # noqa: E402

N = 2048  # sequence length
C = 1024  # model dim
D = 64  # head dim
NH = 4  # heads per core
HD = NH * D  # 256 output channels per core
NCORES = 8
KB = N // 128  # 16 key chunks of 128
QB = N // 512  # 4 query blocks of 512
KC = C // 128  # 8 contraction chunks for projections
SCALE = 1.0 / 32.0  # 1 / sqrt(C)

F32 = mybir.dt.float32
BF16 = mybir.dt.bfloat16
FP16 = mybir.dt.float16


def build_kernel(tc, xt, wqt, wkt, wvt, bias, out_o, out_s):
    nc = tc.nc
    Exp = mybir.ActivationFunctionType.Exp

    with (
        tc.tile_pool(name="res", bufs=1) as res,
        tc.tile_pool(name="ppsum", bufs=2, space="PSUM") as ppsum,
        tc.tile_pool(name="stp", bufs=2, space="PSUM") as stp,
        tc.tile_pool(name="opp", bufs=2, space="PSUM") as opp,
        tc.tile_pool(name="ptp", bufs=16) as ptp,
        tc.tile_pool(name="otp", bufs=2) as otp,
        tc.tile_pool(name="ssp", bufs=2) as ssp,
    ):
        # ---- resident SBUF tensors ----
        # weights laid out [p, pair m, k-chunk, 128 chan] so each pair's
        # half is one contiguous DMA and the first-needed half lands first
        wq_all = res.tile([128, 2, KC, 128], BF16, tag="wq", name="wq")
        wk_all = res.tile([128, 2, KC, 128], BF16, tag="wk", name="wk")
        wv_all = res.tile([128, 2, KC, 128], BF16, tag="wv", name="wv")
        # x laid out quarter-major: one SBUF tile per query-quarter so a
        # consumer's dependency is exactly one contiguous-run DMA
        xt_all = [
            res.tile([128, KC, 512], BF16, tag=f"xt{nb}", name=f"xt{nb}")
            for nb in range(4)
        ]

        def xt_q(k, q0, qw):
            nb, o = divmod(q0, 512)
            return xt_all[nb][:, k, o : o + qw]
        qt_sb = [res.tile([128, N], BF16, tag=f"qt{m}", name=f"qt{m}") for m in range(2)]
        kt_sb = [res.tile([128, N], BF16, tag=f"kt{m}", name=f"kt{m}") for m in range(2)]
        v_sb = [res.tile([128, NH, D], FP16, tag=f"v{kb}", name=f"v{kb}") for kb in range(KB)]
        b_all = res.tile([128, 4], F32, tag="bias", name="bias")
        bq_sb = [b_all[:, m : m + 1] for m in range(2)]
        bk_sb = [b_all[:, 2 + m : 3 + m] for m in range(2)]
        ones_sb = res.tile([128, 1], FP16, tag="ones", name="ones")
        warm_sb = res.tile([1, 2], F32, tag="warm", name="warm")

        # ---- input DMAs, ordered by first use, one issue per logical
        # block (each dma_start costs ~0.6us of serial SP issue time) ----
        nc.sync.dma_start(out=wq_all[:, 0], in_=wqt[:, 0])
        nc.sync.dma_start(out=wk_all[:, 0], in_=wkt[:, 0])
        nc.sync.dma_start(out=xt_all[0][:], in_=xt[0])
        nc.sync.dma_start(out=b_all[:], in_=bias.rearrange("m p -> p m"))

        def dma_rest():
            # issued via the otherwise-idle GpSimd software DGE so their
            # descriptor generation never blocks the SP queue's sync ops
            nc.gpsimd.dma_start(out=wv_all[:], in_=wvt[:])
            nc.gpsimd.dma_start(out=xt_all[1][:], in_=xt[1])
            nc.gpsimd.dma_start(out=xt_all[2][:], in_=xt[2])
            nc.gpsimd.dma_start(out=wq_all[:, 1], in_=wqt[:, 1])
            nc.gpsimd.dma_start(out=wk_all[:, 1], in_=wkt[:, 1])
            nc.gpsimd.dma_start(out=xt_all[3][:], in_=xt[3])

        nc.vector.memset(ones_sb[:], 1.0)
        # warm up the ACT exp table while DMAs land
        nc.vector.memset(warm_sb[:], 0.0)
        nc.scalar.activation(out=warm_sb[:, 0:1], in_=warm_sb[:, 1:2], func=Exp)
        # spin the PE on throwaway matmuls while input DMAs land: ramps the
        # tensor-engine p-state so the real warmup chains run at full rate
        wrm = res.tile([128, 512], FP16, tag="wrm", name="wrm")
        nc.vector.memset(wrm[:], 0.5)
        for i in range(6):
            wps = ppsum.tile([128, 512], F32, tag="qkps", name="wps")
            nc.tensor.matmul(out=wps[0:1, :], lhsT=ones_sb[:], rhs=wrm[:],
                             start=True, stop=True)

        def proj_qk_block(which, m, q0, qw):
            w_all = wq_all if which == "q" else wk_all
            b_sb = (bq_sb if which == "q" else bk_sb)[m]
            t_sb = (qt_sb if which == "q" else kt_sb)[m]
            ps = ppsum.tile([128, 512], F32, tag="qkps", name="qkps")
            for k in range(KC):
                nc.tensor.matmul(
                    out=ps[:128, :qw],
                    lhsT=w_all[:, m, k, :],
                    rhs=xt_q(k, q0, qw),
                    start=(k == 0),
                    stop=(k == KC - 1),
                )
            nc.vector.tensor_scalar_add(
                out=t_sb[:, q0 : q0 + qw], in0=ps[:128, :qw], scalar1=b_sb[:])

        def proj_v_half(pair, kb):
            # one head-pair's 128 channels of V for key chunk kb; bv is
            # folded into the host epilogue (out = O/sum + bv), so a plain
            # copy converts PSUM->fp16
            vps = ppsum.tile([128, 128], F32, tag="qkps", name="vps")
            for k in range(KC):
                nc.tensor.matmul(
                    out=vps[:],
                    lhsT=xt_q(k, kb * 128, 128),
                    rhs=wv_all[:, pair, k, :],
                    start=(k == 0),
                    stop=(k == KC - 1),
                )
            nc.vector.tensor_copy(
                out=v_sb[kb][:, 2 * pair : 2 * pair + 2, :],
                in_=vps[:].rearrange("p (h d) -> p h d", h=2),
            )

        def attn(p, fillers):
            for qb in range(QB):
                qsl = slice(qb * 512, (qb + 1) * 512)
                # both heads' O^T col-packed: head h at partitions h*64..
                o_ps = opp.tile([128, 512], F32, tag="o", name="o")
                # running sums of P^T chunks (softmax denominators): two
                # fp16 parity accumulators keep the DVE in its fast 2-byte
                # mode and halve the accumulation depth.
                ssum = [
                    ssp.tile([128, 2, 512], FP16, tag=f"ssum{j}", name=f"ssum{j}")
                    for j in range(2)
                ]

                def emit_pv(args):
                    kb, pt = args
                    for h in range(2):
                        nc.tensor.matmul(
                            out=o_ps[h * D : (h + 1) * D, :],
                            lhsT=v_sb[kb][:, 2 * p + h, :],
                            rhs=pt[:, h, :],
                            start=(kb == 0),
                            stop=(kb == KB - 1),
                            tile_position=(0, h * D),
                            skip_group_check=True,
                        )
                    sj = ssum[kb % 2]
                    if kb < 2:
                        nc.vector.tensor_copy(out=sj[:], in_=pt[:])
                    else:
                        nc.vector.tensor_add(out=sj[:], in0=sj[:], in1=pt[:])

                # PV + ssum are emitted one kb behind their exp so the
                # in-order PE never sits on the o-psum wait before issuing
                # the next QK pair (which would stall the ACT exp pipeline).
                prev = None
                for kb in range(KB):
                    ksl = slice(kb * 128, (kb + 1) * 128)
                    # st layout [128 keys, head, 512 q] fp32: head h
                    # occupies its own PSUM bank -> the two concurrently-
                    # drained row-packed matmuls hit different banks.
                    st = stp.tile([128, 2, 512], F32, tag="st", name="st")
                    for h in range(2):
                        hsl = slice(h * D, (h + 1) * D)
                        nc.tensor.matmul(
                            out=st[:, h, :],
                            lhsT=kt_sb[p][hsl, ksl],
                            rhs=qt_sb[p][hsl, qsl],
                            start=True,
                            stop=True,
                        )
                    pt = ptp.tile([128, 2, 512], FP16, tag="pt", name="pt")
                    nc.scalar.activation(out=pt[:], in_=st[:], func=Exp, scale=SCALE)
                    for fn in fillers.get((qb, kb), ()):
                        fn()
                    if prev is not None:
                        emit_pv(prev)
                    prev = (kb, pt)
                emit_pv(prev)

                # partition-reduce the running sums with ones-vector
                # matmuls (both parity accumulators accumulate into the same
                # PSUM row); head h lands at PSUM partition 32*h.
                s_ps = sup.tile([33, 512], F32, tag="sps", name="sps")
                for h in range(2):
                    for j in range(2):
                        nc.tensor.matmul(
                            out=s_ps[32 * h : 32 * h + 1, :],
                            lhsT=ones_sb[:],
                            rhs=ssum[j][:, h, :],
                            start=(j == 0),
                            stop=(j == 1),
                            tile_position=(0, 32 * h),
                            skip_group_check=True,
                        )
                ss = otp.tile([33, 512], F32, tag="ss", name="ss")
                for h in range(2):
                    nc.vector.tensor_copy(
                        out=ss[32 * h : 32 * h + 1, :],
                        in_=s_ps[32 * h : 32 * h + 1, :],
                    )
                ss_view = bass.AP(
                    tensor=ss.tensor, offset=ss.offset,
                    ap=[[32 * ss.ap[0][0], 2]] + list(ss.ap[1:]),
                )
                nc.sync.dma_start(out=out_s[p, :, qsl], in_=ss_view)
                ot = otp.tile([128, 512], F32, tag="ot", name="ot")
                nc.vector.tensor_copy(out=ot[:], in_=o_ps[:])
                nc.sync.dma_start(out=out_o[p, :, qsl], in_=ot[:])

        # ---- emission order doubles as scheduler priority; producers must
        # precede consumers.  Warmup computes q/k for pair-0's first query/
        # key block as two clean 8-chains, then the attention loops carry
        # everything else as per-(qb, kb) post-exp filler. ----
        def proj_qk_first():
            for which in ("q", "k"):
                proj_qk_block(which, 0, 0)

        def F(which, m, nb):
            return lambda: proj_qk_block(which, m, nb)

        def V(pair, kb):
            return lambda: proj_v_half(pair, kb)

        # pair-0 filler table: its own V half per chunk in qb0; k blocks
        # just-in-time; pair-1's V half and projections spread over qb1..3
        # Steady-state PE slack behind the exp stream is ~0.37us per chunk;
        # a q/k block costs ~1.83us (needs ~5 chunks of slack around it), a
        # V half-chunk ~0.6us.  qb0 of pair 0 is force-overloaded (V0 + kt
        # just-in-time); everything else is spaced to stay under slack.
        p0 = {}
        for kb in range(KB):
            p0[(0, kb)] = [V(0, kb)]
        p0[(0, 1)].append(F("k", 0, 1))
        p0[(0, 5)].append(F("k", 0, 2))
        p0[(0, 9)].append(F("k", 0, 3))
        p0[(0, 13)].append(F("q", 0, 1))
        p0[(1, 0)] = [F("q", 0, 2)]
        p0[(1, 5)] = [V(1, 0)]
        p0[(1, 7)] = [V(1, 1)]
        p0[(1, 9)] = [V(1, 2)]
        p0[(1, 11)] = [V(1, 3)]
        p0[(1, 13)] = [V(1, 4)]
        p0[(1, 15)] = [V(1, 5)]
        p0[(2, 0)] = [F("q", 0, 3)]
        p0[(2, 6)] = [F("k", 1, 0)]
        p0[(2, 11)] = [V(1, 6)]
        p0[(2, 13)] = [V(1, 7)]
        p0[(2, 15)] = [V(1, 8)]
        p0[(3, 0)] = [F("q", 1, 0)]
        p0[(3, 6)] = [F("k", 1, 1)]
        p0[(3, 9)] = [V(1, 9)]
        p0[(3, 11)] = [V(1, 10)]
        p0[(3, 13)] = [V(1, 11)]
        p0[(3, 15)] = [V(1, 12)]

        p1 = {
            (0, 1): [F("k", 1, 2)],
            (0, 3): [V(1, 13)],
            (0, 5): [V(1, 14)],
            (0, 7): [F("k", 1, 3)],
            (0, 11): [F("q", 1, 1)],
            (0, 13): [V(1, 15)],
            (1, 0): [F("q", 1, 2)],
            (1, 8): [F("q", 1, 3)],
        }

        proj_qk_first()
        attn(0, p0)
        attn(1, p1)


def build_nc():
    nc = bacc.Bacc(
        "TRN2",
        target_bir_lowering=False,
        debug=False,
        num_devices=NCORES,
        enable_partition_id=False,
    )
    xt = nc.dram_tensor("xt", [4, 128, KC, 512], BF16, kind="ExternalInput").ap()
    wqt = nc.dram_tensor("wqt", [128, 2, KC, 128], BF16, kind="ExternalInput").ap()
    wkt = nc.dram_tensor("wkt", [128, 2, KC, 128], BF16, kind="ExternalInput").ap()
    wvt = nc.dram_tensor("wvt", [128, 2, KC, 128], BF16, kind="ExternalInput").ap()
    bias = nc.dram_tensor("bias", [4, 128], F32, kind="ExternalInput").ap()
    out_o = nc.dram_tensor("out_o", [2, 128, N], F32, kind="ExternalOutput").ap()
    out_s = nc.dram_tensor("out_s", [2, 2, N], F32, kind="ExternalOutput").ap()

    with tile.TileContext(nc) as tc:
        build_kernel(tc, xt, wqt, wkt, wvt, bias, out_o, out_s)
    nc.compile()
    return nc


def _w_prep(W, sl):
    # [p, pair m, k-chunk, 128 chan] so each pair half is contiguous
    a = np.ascontiguousarray(np.asarray(W, np.float32)[sl, :].T)  # [C, HD]
    a = a.reshape(KC, 128, 2, 128).transpose(1, 2, 0, 3)
    return np.ascontiguousarray(a).astype(ml_dtypes.bfloat16)


def shard_inputs(inputs):
    x = np.asarray(inputs["x"], np.float32)
    in_maps = []
    for core in range(NCORES):
        b, g = core // 4, core % 4
        sl = slice(g * HD, (g + 1) * HD)
        in_maps.append(
            {
                "xt": np.ascontiguousarray(
                    x[b].T.reshape(KC, 128, 4, 512).transpose(2, 1, 0, 3)
                ).astype(ml_dtypes.bfloat16),
                "wqt": _w_prep(inputs["Wq"], sl),
                "wkt": _w_prep(inputs["Wk"], sl),
                "wvt": _w_prep(inputs["Wv"], sl),
                "bias": np.ascontiguousarray(
                    np.concatenate([
                        np.asarray(inputs["bq"], np.float32)[sl],
                        np.asarray(inputs["bk"], np.float32)[sl],
                    ]).reshape(4, 128)
                ),
            }
        )
    return in_maps


def assemble(results, inputs=None, B=2):
    bv = None if inputs is None else np.asarray(inputs["bv"], np.float32)
    out = np.zeros((B, N, C), np.float32)
    for core in range(NCORES):
        b, g = core // 4, core % 4
        oo = np.asarray(results[core]["out_o"], np.float32)  # [2, 128, N]
        os_ = np.asarray(results[core]["out_s"], np.float32)  # [2, 2, N]
        o = oo.reshape(2, 2, D, N)  # [pair, head, d, n]
        on = o / os_[:, :, None, :]
        # [pair, head, d, n] -> [n, pair*2*D + head*D + d]
        blk = on.transpose(3, 0, 1, 2).reshape(N, HD)
        if bv is not None:
            blk = blk + bv[g * HD : (g + 1) * HD][None, :]
        out[b, :, g * HD : (g + 1) * HD] = blk
    return out


_NC_CACHE = None


def _get_nc():
    global _NC_CACHE
    if _NC_CACHE is None:
        _NC_CACHE = build_nc()
    return _NC_CACHE


def kernel(**inputs):
    nc = _get_nc()
    in_maps = shard_inputs(inputs)
    res = run_bass_kernel_spmd(
        nc,
        in_maps,
        core_ids=list(range(NCORES)),
        trace=bool(int(os.environ.get("KERNEL_TRACE", "0"))),
    )
    return assemble(res.results, inputs=inputs, B=int(np.asarray(inputs["x"]).shape[0]))
